# revision 24
# baseline (speedup 1.0000x reference)
"""Trainium2 Bass kernel for DiffusionProteinFuncModel loss.

Sharding: data-parallel over batch B (4 per core) for q_sample + MHA + MSE;
channel-parallel over D (256 per core) for the per-channel contrastive
losses. Each core emits 4 partial sums; host combines into the scalar loss.

Self-contained: hardcodes all shapes; builds the Bass program once and runs
it on 8 NeuronCores via run_bass_kernel_spmd.
"""

import numpy as np

import bass_rust
import concourse.bass as bass
import concourse.bacc as bacc
import concourse.mybir as mybir
from concourse.tile import TileContext
from concourse.bass_utils import run_bass_kernel_spmd
from concourse.masks import make_identity

# Problem constants
B, LS, LL, D, H, T = 32, 256, 256, 2048, 16, 1000
TAU = 0.07
SEQ = LS + LL          # 512
DH = D // H            # 128
P = 128
KO = D // P            # 16 partition blocks of the model dim
NCORES = 8
BL = B // NCORES       # 4 batches per core
CHL = D // NCORES      # 256 contrastive channels per core
TB = SEQ // P          # 4 token blocks per batch
NG = CHL // 2          # 128 two-channel contrastive groups
CCH = 16               # channels per pre-scale chunk
NCHUNK = CHL // CCH    # 16
ISQ = 1.0 / np.sqrt(DH).astype(np.float32)   # attention scale
TAU2 = TAU * TAU

F32 = mybir.dt.float32
BF16 = mybir.dt.bfloat16
AX = bass_rust.AxisListType.X
NLT = float(-np.log(TAU))   # -ln(tau), bias for rowscale exp


def build_bass():
    nc = bacc.Bacc("TRN2", target_bir_lowering=False, debug=False,
                   enable_asserts=False)

    esT = nc.dram_tensor("esT", [BL, D, LS], F32, kind="ExternalInput")
    elT = nc.dram_tensor("elT", [BL, D, LL], F32, kind="ExternalInput")
    nsT = nc.dram_tensor("nsT", [BL, D, SEQ], F32, kind="ExternalInput")
    wq_d = nc.dram_tensor("wq", [D, D], F32, kind="ExternalInput")
    wk_d = nc.dram_tensor("wk", [D, D], F32, kind="ExternalInput")
    wv_d = nc.dram_tensor("wv", [D, D], F32, kind="ExternalInput")
    wo_d = nc.dram_tensor("wo", [D, D], F32, kind="ExternalInput")
    sa_d = nc.dram_tensor("sa", [BL, 1], F32, kind="ExternalInput")
    s1m_d = nc.dram_tensor("s1m", [BL, 1], F32, kind="ExternalInput")
    gT_d = nc.dram_tensor("gT", [LS, CHL, 2, B], F32, kind="ExternalInput")
    eyeM_d = nc.dram_tensor("eyeM", [P, P], F32, kind="ExternalInput")
    eyeX_d = nc.dram_tensor("eyeX", [P, P], F32, kind="ExternalInput")
    mask_d = nc.dram_tensor("maskmat", [P, 4], F32, kind="ExternalInput")
    pout = nc.dram_tensor("pout", [4, 4], F32, kind="ExternalOutput")

    wq_r = wq_d.ap().rearrange("(ko p) n -> p ko n", p=P)
    wk_r = wk_d.ap().rearrange("(ko p) n -> p ko n", p=P)
    wv_r = wv_d.ap().rearrange("(ko p) n -> p ko n", p=P)
    wo_r = wo_d.ap().rearrange("(ko p) n -> p ko n", p=P)

    AF = mybir.ActivationFunctionType
    OP = mybir.AluOpType

    with TileContext(nc) as tc:
        with (
            tc.tile_pool(name="cst", bufs=1) as cst,
            tc.tile_pool(name="acc", bufs=1) as accp,
        ):
            ones_col = cst.tile([P, 1], BF16)
            nc.any.memset(ones_col[:], 1.0)
            nlt_t = cst.tile([P, 1], F32)
            nc.any.memset(nlt_t[:], NLT)
            ones_row = cst.tile([1, P], BF16)
            nc.any.memset(ones_row[:], 1.0)
            ident_bf = cst.tile([P, P], BF16)
            make_identity(nc, ident_bf[:])
            eyeM_sb = cst.tile([P, P], F32)
            nc.sync.dma_start(eyeM_sb[:], eyeM_d.ap())
            eyeX_sb = cst.tile([P, P], F32)
            nc.sync.dma_start(eyeX_sb[:], eyeX_d.ap())
            mask_sb = cst.tile([P, 4], F32)
            nc.sync.dma_start(mask_sb[:], mask_d.ap())

            xsq_acc = accp.tile([P, BL * KO], F32)
            mse_acc = accp.tile([P, BL * KO], F32)
            esum_acc = accp.tile([P, NG], F32)
            d1_acc = accp.tile([P, NG], F32)
            d2_acc = accp.tile([P, NG], F32)
            dm_all = accp.tile([P, NG], F32)
            for t in (xsq_acc, mse_acc, esum_acc, d1_acc, d2_acc, dm_all):
                nc.any.memset(t[:], 0.0)

            # ---------------- Phase A+B+C: q_sample, MHA, MSE ----------------
            with tc.tile_pool(name="big", bufs=1) as big:
                xt_bf = big.tile([P, BL, KO, SEQ], BF16)     # x_t^T, din-major
                ao_bf = big.tile([P, BL, KO, SEQ], BF16)     # attnout^T, (h,dh)-major

                # Phase A: x_t = sa*x_start + s1m*noise (transposed layout),
                # plus sum(x_start^2) partials.
                with tc.tile_pool(name="pA", bufs=3) as pA:
                    for b in range(BL):
                        sa_t = pA.tile([P, 1], F32, tag="sab")
                        nc.sync.dma_start(sa_t[:], sa_d.ap()[b, :].to_broadcast((P, 1)))
                        s1_t = pA.tile([P, 1], F32, tag="s1b")
                        nc.sync.dma_start(s1_t[:], s1m_d.ap()[b, :].to_broadcast((P, 1)))
                        for ko in range(KO):
                            es_t = pA.tile([P, LS], F32, tag="es")
                            nc.sync.dma_start(es_t[:], esT.ap()[b, ko * P:(ko + 1) * P, :])
                            el_t = pA.tile([P, LL], F32, tag="el")
                            nc.sync.dma_start(el_t[:], elT.ap()[b, ko * P:(ko + 1) * P, :])
                            ns_t = pA.tile([P, SEQ], F32, tag="ns")
                            nc.sync.dma_start(ns_t[:], nsT.ap()[b, ko * P:(ko + 1) * P, :])
                            tmp = pA.tile([P, SEQ], F32, tag="tmp")
                            nc.vector.tensor_scalar_mul(tmp[:], ns_t[:], s1_t[:])
                            nc.vector.scalar_tensor_tensor(
                                xt_bf[:, b, ko, 0:LS], es_t[:], sa_t[:],
                                tmp[:, 0:LS], OP.mult, OP.add)
                            nc.vector.scalar_tensor_tensor(
                                xt_bf[:, b, ko, LS:SEQ], el_t[:], sa_t[:],
                                tmp[:, LS:SEQ], OP.mult, OP.add)

                # Phase B: per-head attention, batches inner.
                with (
                    tc.tile_pool(name="pW", bufs=1) as pW,
                    tc.tile_pool(name="pWb", bufs=1) as pWb,
                    tc.tile_pool(name="pQ", bufs=2) as pQ,
                    tc.tile_pool(name="pE", bufs=2) as pE,
                    tc.tile_pool(name="psA", bufs=2, space="PSUM") as psA,
                    tc.tile_pool(name="psS", bufs=1, space="PSUM") as psS,
                    tc.tile_pool(name="psR", bufs=1, space="PSUM") as psR,
                    tc.tile_pool(name="psB", bufs=1, space="PSUM") as psB,
                    tc.tile_pool(name="psT", bufs=1, space="PSUM") as psT,
                    tc.tile_pool(name="psO", bufs=1, space="PSUM") as psO,
                ):
                    for h in range(H):
                        hs = slice(h * P, (h + 1) * P)
                        wq_f = pW.tile([P, KO, P], F32, tag="wf")
                        nc.sync.dma_start(wq_f[:], wq_r[:, :, hs])
                        wq_b = pWb.tile([P, KO, P], BF16, tag="wqb")
                        nc.vector.tensor_copy(wq_b[:], wq_f[:])
                        wk_f = pW.tile([P, KO, P], F32, tag="wf")
                        nc.sync.dma_start(wk_f[:], wk_r[:, :, hs])
                        wk_b = pWb.tile([P, KO, P], BF16, tag="wkb")
                        nc.vector.tensor_copy(wk_b[:], wk_f[:])
                        wv_f = pW.tile([P, KO, P], F32, tag="wf")
                        nc.sync.dma_start(wv_f[:], wv_r[:, :, hs])
                        wv_b = pWb.tile([P, KO, P], BF16, tag="wvb")
                        nc.vector.tensor_copy(wv_b[:], wv_f[:])

                        for b in range(BL):
                            # Q^T, K^T, V^T : [dh, tok]
                            psq = psA.tile([P, SEQ], F32, tag="psA")
                            for ko in range(KO):
                                nc.tensor.matmul(psq[:], wq_b[:, ko, :], xt_bf[:, b, ko, :],
                                                 start=(ko == 0), stop=(ko == KO - 1))
                            qT = pQ.tile([P, SEQ], BF16, tag="qT")
                            nc.vector.tensor_copy(qT[:], psq[:])

                            psk = psA.tile([P, SEQ], F32, tag="psA")
                            for ko in range(KO):
                                nc.tensor.matmul(psk[:], wk_b[:, ko, :], xt_bf[:, b, ko, :],
                                                 start=(ko == 0), stop=(ko == KO - 1))
                            kT = pQ.tile([P, SEQ], BF16, tag="kT")
                            nc.scalar.copy(kT[:], psk[:])

                            psv = psA.tile([P, SEQ], F32, tag="psA")
                            for ko in range(KO):
                                nc.tensor.matmul(psv[:], wv_b[:, ko, :], xt_bf[:, b, ko, :],
                                                 start=(ko == 0), stop=(ko == KO - 1))
                            vT = pQ.tile([P, SEQ], BF16, tag="vT")
                            nc.scalar.copy(vT[:], psv[:])

                            # V natural via PE transpose
                            v_bf = pQ.tile([P, TB, P], BF16, tag="vn")
                            for tb in range(TB):
                                pst = psT.tile([P, P], BF16, tag="pst")
                                nc.tensor.transpose(pst[:], vT[:, tb * P:(tb + 1) * P],
                                                    ident_bf[:])
                                nc.vector.tensor_copy(v_bf[:, tb, :], pst[:])

                            # E^T = exp(S^T / sqrt(dh)) : [ktok, q]
                            eT = pE.tile([P, TB, SEQ], BF16, tag="eT")
                            for half in range(2):
                                pss = psS.tile([P, 2, SEQ], F32, tag="psS")
                                for j in range(2):
                                    kb = 2 * half + j
                                    nc.tensor.matmul(pss[:, j, :],
                                                     kT[:, kb * P:(kb + 1) * P], qT[:],
                                                     start=True, stop=True)
                                nc.scalar.activation(eT[:, 2 * half:2 * half + 2, :],
                                                     pss[:], AF.Exp, scale=float(ISQ))

                            # softmax denominator r[q] then 1/r broadcast
                            psr = psR.tile([1, SEQ], F32, tag="psr")
                            for kb in range(TB):
                                nc.tensor.matmul(psr[:], ones_col[:], eT[:, kb, :],
                                                 start=(kb == 0), stop=(kb == TB - 1))
                            rcp_f = pQ.tile([1, SEQ], F32, tag="rcpf")
                            nc.vector.reciprocal_approx_fast(rcp_f[:], psr[:])
                            rcp = pQ.tile([1, SEQ], BF16, tag="rcp")
                            nc.vector.tensor_copy(rcp[:], rcp_f[:])
                            psb = psB.tile([P, SEQ], F32, tag="psb")
                            nc.tensor.matmul(psb[:], ones_row[:], rcp[:],
                                             start=True, stop=True)
                            bc_sb = pQ.tile([P, SEQ], BF16, tag="bcs")
                            nc.vector.tensor_copy(bc_sb[:], psb[:])

                            # out2^T = (E @ V)^T, normalized
                            pso = psO.tile([P, SEQ], F32, tag="pso")
                            for kb in range(TB):
                                nc.tensor.matmul(pso[:], v_bf[:, kb, :], eT[:, kb, :],
                                                 start=(kb == 0), stop=(kb == TB - 1))
                            nc.vector.tensor_mul(ao_bf[:, b, h, :], pso[:], bc_sb[:])

                # Phase C: mo^T = Wo-proj(attnout^T); accumulate (x_start-mo)^2
                with (
                    tc.tile_pool(name="pWo", bufs=2) as pWo,
                    tc.tile_pool(name="pX", bufs=3) as pX,
                    tc.tile_pool(name="psC", bufs=2, space="PSUM") as psC,
                ):
                    for do in range(KO):
                        wo_f = pWo.tile([P, KO, P], F32, tag="wof")
                        nc.sync.dma_start(wo_f[:], wo_r[:, :, do * P:(do + 1) * P])
                        wo_b = pWo.tile([P, KO, P], BF16, tag="wob")
                        nc.vector.tensor_copy(wo_b[:], wo_f[:])
                        for b in range(BL):
                            psm = psC.tile([P, SEQ], F32, tag="psm")
                            for ko in range(KO):
                                nc.tensor.matmul(psm[:], wo_b[:, ko, :], ao_bf[:, b, ko, :],
                                                 start=(ko == 0), stop=(ko == KO - 1))
                            xs_t = pX.tile([P, SEQ], F32, tag="xs")
                            nc.sync.dma_start(xs_t[:, 0:LS], esT.ap()[b, do * P:(do + 1) * P, :])
                            nc.sync.dma_start(xs_t[:, LS:SEQ], elT.ap()[b, do * P:(do + 1) * P, :])
                            d_t = pX.tile([P, SEQ], F32, tag="df")
                            nc.vector.tensor_tensor(d_t[:], xs_t[:], psm[:], OP.subtract)
                            dmc = pX.tile([P, SEQ], BF16, tag="dmc")
                            col = do * BL + b
                            nc.scalar.activation(dmc[:], d_t[:], AF.Square,
                                                 accum_out=mse_acc[:, col:col + 1])
                            xq_t = pX.tile([P, SEQ], F32, tag="xq")
                            nc.vector.tensor_tensor(xq_t[:], xs_t[:], xs_t[:], OP.mult)
                            nc.vector.reduce_sum(xsq_acc[:, col:col + 1], xq_t[:], axis=AX)

            # ---------------- Phase D: per-channel contrastive ----------------
            with (
                tc.tile_pool(name="pGn", bufs=1) as pGn,
                tc.tile_pool(name="pG", bufs=2) as pG,
                tc.tile_pool(name="pD", bufs=3) as pD,
                tc.tile_pool(name="psN", bufs=2, space="PSUM") as psN,
                tc.tile_pool(name="psB2", bufs=2, space="PSUM") as psB2,
                tc.tile_pool(name="psG", bufs=3, space="PSUM") as psG,
            ):
                # gn layout: [p, lo, ch, {fl,fs}, b]; fl pre-scaled by 1/||fl||
                gn_bf = pGn.tile([P, 2, CHL, 2, B], BF16)
                for cc in range(NCHUNK):
                    cs = slice(cc * CCH, (cc + 1) * CCH)
                    gf = pG.tile([P, 2, CCH, 2, B], F32, tag="gf")
                    nc.sync.dma_start(
                        gf[:], gT_d.ap()[:, cs, :, :].rearrange(
                            "(lo p) c t b -> p lo c t b", p=P))
                    flsq = pG.tile([P, 2, CCH, B], BF16, tag="flsq")
                    nc.scalar.activation(flsq[:], gf[:, :, :, 0, :], AF.Square)
                    psn = psN.tile([1, CCH * B], F32, tag="psn")
                    for lo in range(2):
                        nc.tensor.matmul(psn[:],
                                         ones_col[:],
                                         flsq[:, lo].rearrange("p c b -> p (c b)"),
                                         start=(lo == 0), stop=(lo == 1))
                    # 1/||fl|| = exp(-0.5*ln(||fl||^2)) — keeps ACT on one table set
                    lnn = pG.tile([1, CCH * B], F32, tag="lnn")
                    nc.scalar.activation(lnn[:], psn[:], AF.Ln)
                    inv2 = pG.tile([1, CCH * B], BF16, tag="inv2")
                    nc.scalar.activation(inv2[:], lnn[:], AF.Exp, scale=-0.5)
                    psb2 = psB2.tile([P, CCH * B], F32, tag="psb2")
                    nc.tensor.matmul(psb2[:], ones_row[:], inv2[:], start=True, stop=True)
                    for lo in range(2):
                        nc.vector.tensor_tensor(
                            gn_bf[:, lo, cs, 0, :], gf[:, lo, :, 0, :],
                            psb2[:].rearrange("p (c b) -> p c b", c=CCH), OP.mult)
                        nc.vector.tensor_copy(gn_bf[:, lo, cs, 1, :], gf[:, lo, :, 1, :])

                gn0 = gn_bf[:, 0].rearrange("p c t b -> p (c t b)")
                gn1 = gn_bf[:, 1].rearrange("p c t b -> p (c t b)")
                # pass 1: Gram diagonals -> dm_all
                for g in range(NG):
                    gs = slice(g * P, (g + 1) * P)
                    psg = psG.tile([P, P], F32, tag="psg")
                    nc.tensor.matmul(psg[:], gn0[:, gs], gn0[:, gs], start=True, stop=False)
                    nc.tensor.matmul(psg[:], gn1[:, gs], gn1[:, gs], start=False, stop=True)
                    dmpg = pD.tile([P, P], F32, tag="dmpg")
                    nc.vector.tensor_tensor(dmpg[:], psg[:], eyeM_sb[:], OP.mult)
                    nc.vector.reduce_sum(dm_all[:, g:g + 1], dmpg[:], axis=AX)
                # bulk: rowscale = 1/(TAU*sqrt(dm)) = exp(-0.5*ln(dm) - ln(TAU))
                lnd = pD.tile([P, NG], F32, tag="lnd")
                nc.scalar.activation(lnd[:], dm_all[:], AF.Ln)
                rs_all = pD.tile([P, NG], F32, tag="rs_all")
                nc.scalar.activation(rs_all[:], lnd[:], AF.Exp, scale=-0.5,
                                     bias=nlt_t[:])
                nc.vector.tensor_tensor(d2_acc[:], dm_all[:], rs_all[:], OP.mult)
                # pass 2: recompute Gram, scale rows, extract cross-diag + lse sums
                for g in range(NG):
                    gs = slice(g * P, (g + 1) * P)
                    psg = psG.tile([P, P], F32, tag="psg")
                    nc.tensor.matmul(psg[:], gn0[:, gs], gn0[:, gs], start=True, stop=False)
                    nc.tensor.matmul(psg[:], gn1[:, gs], gn1[:, gs], start=False, stop=True)
                    lg = pD.tile([P, P], F32, tag="lg")
                    nc.vector.tensor_scalar_mul(lg[:], psg[:], rs_all[:, g:g + 1])
                    dmpx = pD.tile([P, P], F32, tag="dmpx")
                    nc.vector.tensor_tensor(dmpx[:], lg[:], eyeX_sb[:], OP.mult)
                    nc.vector.reduce_sum(d1_acc[:, g:g + 1], dmpx[:], axis=AX)
                    ed = pD.tile([P, 32], BF16, tag="ed")
                    nc.scalar.activation(ed[0:64, :], lg[0:64, 0:32], AF.Exp,
                                         accum_out=esum_acc[0:64, g:g + 1])
                    nc.scalar.activation(ed[64:128, :], lg[64:128, 64:96], AF.Exp,
                                         accum_out=esum_acc[64:128, g:g + 1])

            # ---------------- Final reduction ----------------
            with (
                tc.tile_pool(name="pF", bufs=1) as pF,
                tc.tile_pool(name="psF", bufs=1, space="PSUM") as psF,
            ):
                lse_t = pF.tile([P, NG], F32)
                nc.scalar.activation(lse_t[:], esum_acc[:], AF.Ln)
                r_t = pF.tile([P, NG], F32)
                nc.vector.tensor_tensor(r_t[:], lse_t[:], d1_acc[:], OP.subtract)
                nc.vector.tensor_tensor(r_t[0:32, :], r_t[0:32, :], d2_acc[0:32, :],
                                        OP.subtract)
                nc.vector.tensor_tensor(r_t[64:96, :], r_t[64:96, :], d2_acc[64:96, :],
                                        OP.subtract)
                colmat = pF.tile([P, 4], F32)
                nc.vector.reduce_sum(colmat[:, 0:1], xsq_acc[:], axis=AX)
                nc.vector.reduce_sum(colmat[:, 1:2], mse_acc[:], axis=AX)
                mcol = pF.tile([P, 1], F32)
                nc.vector.reduce_sum(mcol[:], r_t[:], axis=AX)
                nc.vector.tensor_copy(colmat[:, 2:3], mcol[:])
                nc.vector.tensor_copy(colmat[:, 3:4], mcol[:])
                psf = psF.tile([4, 4], F32)
                nc.tensor.matmul(psf[:], mask_sb[:], colmat[:], start=True, stop=True)
                out_sb = pF.tile([4, 4], F32)
                nc.scalar.copy(out_sb[:], psf[:])
                nc.sync.dma_start(pout.ap()[:, :], out_sb[:])

    nc.compile()
    return nc


_NC_CACHE = {}


def get_nc():
    if "nc" not in _NC_CACHE:
        _NC_CACHE["nc"] = build_bass()
    return _NC_CACHE["nc"]


def make_core_inputs(embed_seq, embed_label, noise, sqrt_alphas_cumprod,
                     sqrt_one_minus_alphas_cumprod, Wq, Wk, Wv, Wo, timestep):
    eyeM = np.eye(P, dtype=np.float32)
    eyeX = np.zeros((P, P), dtype=np.float32)
    for i in range(32):
        eyeX[32 + i, i] = 1.0
        eyeX[96 + i, 64 + i] = 1.0
    maskmat = np.zeros((P, 4), dtype=np.float32)
    maskmat[:, 0] = 1.0
    maskmat[:, 1] = 1.0
    maskmat[32:64, 2] = 1.0
    maskmat[96:128, 2] = 1.0
    maskmat[0:32, 3] = 1.0
    maskmat[64:96, 3] = 1.0

    sa_all = np.asarray(sqrt_alphas_cumprod)[np.asarray(timestep)].astype(np.float32)
    s1m_all = np.asarray(sqrt_one_minus_alphas_cumprod)[np.asarray(timestep)].astype(np.float32)

    es = np.asarray(embed_seq, dtype=np.float32)
    el = np.asarray(embed_label, dtype=np.float32)
    ns = np.asarray(noise, dtype=np.float32)

    in_maps = []
    for c in range(NCORES):
        bsl = slice(c * BL, (c + 1) * BL)
        chsl = slice(c * CHL, (c + 1) * CHL)
        flT = np.ascontiguousarray(el[:, :, chsl].transpose(1, 2, 0))  # [L, CHL, B]
        fsT = np.ascontiguousarray(es[:, :, chsl].transpose(1, 2, 0))
        gT = np.ascontiguousarray(np.stack([flT, fsT], axis=2))        # [L, CHL, 2, B]
        in_maps.append({
            "esT": np.ascontiguousarray(es[bsl].transpose(0, 2, 1)),
            "elT": np.ascontiguousarray(el[bsl].transpose(0, 2, 1)),
            "nsT": np.ascontiguousarray(ns[bsl].transpose(0, 2, 1)),
            "wq": np.ascontiguousarray(np.asarray(Wq, dtype=np.float32)),
            "wk": np.ascontiguousarray(np.asarray(Wk, dtype=np.float32)),
            "wv": np.ascontiguousarray(np.asarray(Wv, dtype=np.float32)),
            "wo": np.ascontiguousarray(np.asarray(Wo, dtype=np.float32)),
            "sa": sa_all[bsl].reshape(BL, 1).copy(),
            "s1m": s1m_all[bsl].reshape(BL, 1).copy(),
            "gT": gT,
            "eyeM": eyeM,
            "eyeX": eyeX,
            "maskmat": maskmat,
        })
    return in_maps


def combine_partials(partials, sqrt_alphas_cumprod):
    """partials: list of 8 [4,4] arrays; diag = [xsq, mse, match, ctr] sums."""
    xsq = sum(float(np.asarray(p)[0, 0]) for p in partials)
    mse = sum(float(np.asarray(p)[1, 1]) for p in partials)
    match = sum(float(np.asarray(p)[2, 2]) for p in partials)
    ctr = sum(float(np.asarray(p)[3, 3]) for p in partials)
    n_el = B * SEQ * D
    sa_T = float(np.asarray(sqrt_alphas_cumprod)[T - 1])
    loss = mse / n_el + (sa_T ** 2) * xsq / n_el + match / (D * B) + ctr / (D * B)
    return np.float32(loss)


def kernel(**inputs):
    nc = get_nc()
    in_maps = make_core_inputs(**inputs)
    res = run_bass_kernel_spmd(nc, in_maps, core_ids=list(range(NCORES)))
    partials = [res.results[c]["pout"] for c in range(NCORES)]
    return combine_partials(partials, inputs["sqrt_alphas_cumprod"])
